# revision 6
# baseline (speedup 1.0000x reference)
"""Trainium2 Bass kernel for the FCR contrastive FFT loss.

Math: the loss only needs d = mean |FFT2(x)| for difference images
x = a-p, a-n[j0], a-n[j1] (FFT linearity).  Each 2D FFT of a real
256x256 image is computed as two DFT matmul stages on the tensor
engine, using conjugate symmetry to halve the work: only frequency
rows u=0..128 are computed, with row weights w = [1, 2, ..., 2, 1]
folded into the stage-1 DFT constants.  Full-plane sum of |Z| equals
the weighted sum over the half-plane.

Sharding: data-parallel over batch B=32 -> 4 items (36 diff images)
per NeuronCore.  Each core outputs per-image partial sums [128, 72]
(128 partitions x 2 v-chunks per image); the host finishes the tiny
reduction and the contrastive combine.
"""

import numpy as np

import concourse.bass as bass
import concourse.tile as tile
from concourse import bacc, mybir
from concourse.bass_utils import run_bass_kernel_spmd

B, C, H, W = 32, 3, 256, 256
NCORES = 8
BL = B // NCORES          # batch items per core
NIMG = BL * C * 3         # 36 difference images per core
NU = 130                  # u columns: u=0..128 real + 1 zero pad col (even width)
NF = 2 * NU               # [real | imag] packed free dim
EPS = 1e-7
F16 = mybir.dt.float16
F32 = mybir.dt.float32


def _build_consts():
    """DFT constant matrices (fp16) with symmetry weights folded in."""
    r = np.arange(H, dtype=np.float64)
    u = np.arange(NU, dtype=np.float64)
    w = np.ones(NU)
    w[1:128] = 2.0
    w[129] = 0.0                      # dummy pad column
    ang1 = 2.0 * np.pi * np.outer(r, u) / H       # [row, u]
    frw = np.cos(ang1) * w
    fiw = -np.sin(ang1) * w
    frw[:, 129] = 0.0
    fiw[:, 129] = 0.0
    fab = np.concatenate([frw, fiw], axis=1)      # [256, 260]
    fab = fab.reshape(2, 128, NF).astype(np.float16)

    ang2 = 2.0 * np.pi * np.outer(r, r) / H       # [col, v]
    fr2 = np.cos(ang2).reshape(2, 128, W).astype(np.float16)
    fi2 = (-np.sin(ang2)).reshape(2, 128, W).astype(np.float16)
    return fab, fr2, fi2


def _build_program(reps=1):
    """Bass/Tile program: 36 images x (2-stage DFT matmul + |Z| reduce).

    reps>1 repeats the whole image loop (identical output) — used only
    by the benchmark harness to measure steady-state per-rep HW time.
    """
    nc = bacc.Bacc("TRN2", debug=False, num_devices=NCORES)
    x_d = nc.dram_tensor("x", [NIMG, 2, 128, W], F16, kind="ExternalInput").ap()
    fab_d = nc.dram_tensor("fab", [2, 128, NF], F16, kind="ExternalInput").ap()
    fr2_d = nc.dram_tensor("fr2", [2, 128, W], F16, kind="ExternalInput").ap()
    fi2_d = nc.dram_tensor("fi2", [2, 128, W], F16, kind="ExternalInput").ap()
    out_d = nc.dram_tensor("out", [128, 2 * NIMG], F32, kind="ExternalOutput").ap()

    with tile.TileContext(nc) as tc:
        with (
            tc.tile_pool(name="consts", bufs=1) as cpool,
            tc.tile_pool(name="xin", bufs=3) as xpool,
            tc.tile_pool(name="ysb", bufs=3) as ypool,
            tc.tile_pool(name="tmp", bufs=3) as tpool,
            tc.tile_pool(name="accp", bufs=1) as apool,
            tc.tile_pool(name="py", bufs=2, space="PSUM") as pypool,
            tc.tile_pool(name="pz", bufs=2, space="PSUM") as pzpool,
        ):
            fab = [cpool.tile([128, NF], F16, tag=f"fab{k}", name=f"fab{k}") for k in range(2)]
            fr2 = [cpool.tile([128, W], F16, tag=f"fr2{k}", name=f"fr2{k}") for k in range(2)]
            fi2 = [cpool.tile([128, W], F16, tag=f"fi2{k}", name=f"fi2{k}") for k in range(2)]
            for k in range(2):
                nc.sync.dma_start(out=fab[k], in_=fab_d[k])
                nc.sync.dma_start(out=fr2[k], in_=fr2_d[k])
                nc.sync.dma_start(out=fi2[k], in_=fi2_d[k])
            acc = apool.tile([128, 2 * NIMG], F32)

            for j in [jj for _ in range(reps) for jj in range(NIMG)]:
                xs = [xpool.tile([128, W], F16, tag=f"x{k}", name=f"x{k}") for k in range(2)]
                for k in range(2):
                    nc.sync.dma_start(out=xs[k], in_=x_d[j, k])

                # Stage 1: yT[col, u] = sum_row x[row, col] * F[row, u]
                # y psum = [yr | yi], built per col-chunk m.
                ysb, ysb2 = [], []
                for m in range(2):
                    y = pypool.tile([128, NF], F32, tag=f"y{m}")
                    for k in range(2):
                        nc.tensor.matmul(
                            y,
                            lhsT=xs[k][:, m * 128 : (m + 1) * 128],
                            rhs=fab[k],
                            start=(k == 0),
                            stop=(k == 1),
                        )
                    ys = ypool.tile([128, NF], F16, tag=f"ysb{m}")
                    nc.vector.tensor_copy(ys, y)          # fp32 -> fp16 cast
                    ys2 = ypool.tile([128, NF], F16, tag=f"ysb2{m}")
                    nc.vector.tensor_scalar_mul(ys2[:, 0:NU], ys[:, NU:NF], -1.0)
                    nc.vector.tensor_copy(ys2[:, NU:NF], ys[:, 0:NU])
                    ysb.append(ys)
                    ysb2.append(ys2)

                # Stage 2: zT[v, u] = sum_col F[col, v] * y[col, u]
                # z psum = [zr | zi]:
                #   lhsT=Fr, rhs=[yr|yi]   -> [Fr.yr | Fr.yi]
                #   lhsT=Fi, rhs=[-yi|yr]  -> [-Fi.yi | Fi.yr]
                for v in range(2):
                    z = pzpool.tile([128, NF], F32, tag=f"z{v}")
                    vs = slice(v * 128, (v + 1) * 128)
                    nc.tensor.matmul(z, lhsT=fr2[0][:, vs], rhs=ysb[0],
                                     start=True, stop=False)
                    nc.tensor.matmul(z, lhsT=fr2[1][:, vs], rhs=ysb[1],
                                     start=False, stop=False)
                    nc.tensor.matmul(z, lhsT=fi2[0][:, vs], rhs=ysb2[0],
                                     start=False, stop=False)
                    nc.tensor.matmul(z, lhsT=fi2[1][:, vs], rhs=ysb2[1],
                                     start=False, stop=True)

                    # |Z| = sqrt(zr^2 + zi^2), summed over u into acc column
                    t = tpool.tile([128, NF], F32, tag=f"t{v}")
                    nc.scalar.activation(
                        out=t, in_=z, func=mybir.ActivationFunctionType.Square,
                    )
                    s = tpool.tile([128, NU], F32, tag=f"s{v}")
                    nc.vector.scalar_tensor_tensor(
                        out=s,
                        in0=t[:, 0:NU],
                        scalar=1.0,
                        in1=t[:, NU:NF],
                        op0=mybir.AluOpType.mult,
                        op1=mybir.AluOpType.add,
                    )
                    sq = tpool.tile([128, NU], F32, tag=f"sq{v}")
                    nc.scalar.activation(
                        out=sq,
                        in_=s,
                        func=mybir.ActivationFunctionType.Sqrt,
                        accum_out=acc[:, 2 * j + v : 2 * j + v + 1],
                    )

            nc.sync.dma_start(out=out_d, in_=acc)

    nc.compile()
    return nc


_CACHE = {}


def _get_program():
    if "nc" not in _CACHE:
        _CACHE["nc"] = _build_program()
    return _CACHE["nc"]


def _prep_core_inputs(a, p, n, neg_idx, fab, fr2, fi2):
    """Shard over batch; build the 36 fp16 difference images per core."""
    neg = neg_idx.astype(np.int64)
    in_maps = []
    for c in range(NCORES):
        sl = slice(c * BL, (c + 1) * BL)
        ash = a[sl]                                   # [BL, C, H, W]
        d0 = ash - p[sl]
        d1 = ash - n[neg[sl, 0]]
        d2 = ash - n[neg[sl, 1]]
        diffs = np.stack([d0, d1, d2], axis=2)        # [BL, C, 3, H, W]
        x = diffs.reshape(NIMG, 2, 128, W).astype(np.float16)
        in_maps.append({"x": x, "fab": fab, "fr2": fr2, "fi2": fi2})
    return in_maps


def run(a, p, n, neg_idx, trace=False, trace_kwargs=None):
    """Run on 8 cores; returns (scalar_output, BassKernelResults)."""
    nc = _get_program()
    fab, fr2, fi2 = _build_consts()
    in_maps = _prep_core_inputs(
        np.asarray(a, dtype=np.float32),
        np.asarray(p, dtype=np.float32),
        np.asarray(n, dtype=np.float32),
        np.asarray(neg_idx),
        fab, fr2, fi2,
    )
    res = run_bass_kernel_spmd(
        nc, in_maps, core_ids=list(range(NCORES)),
        trace=trace, **(trace_kwargs or {}),
    )

    total = 0.0
    norm = float(C * H * W)
    for c in range(NCORES):
        acc = res.results[c]["out"]                   # [128, 72] fp32
        sums = acc.sum(axis=0, dtype=np.float64)      # [72]
        s_img = sums[0::2] + sums[1::2]               # [36]
        d = s_img.reshape(BL, C, 3).sum(axis=1) / norm  # [BL, 3]
        d_ap, d_an0, d_an1 = d[:, 0], d[:, 1], d[:, 2]
        total += (d_ap / (d_an0 + EPS) + d_ap / (d_an1 + EPS)).sum()
    out = np.float32(total / (2 * B))
    return np.asarray(out, dtype=np.float32), res


def kernel(a, p, n, neg_idx):
    out, _ = run(a, p, n, neg_idx)
    return out


# revision 10
# speedup vs baseline: 110.4915x; 110.4915x over previous
"""Trainium2 Bass kernel for the FCR contrastive FFT loss.

Math: the loss only needs d = mean |FFT2(x)| for difference images
x = a-p, a-n[j0], a-n[j1] (FFT linearity).  Each 2D FFT of a real
256x256 image is computed as two DFT matmul stages on the tensor
engine, using conjugate symmetry to halve the work: only frequency
rows u=0..128 are computed, with row weights w = [1, 2, ..., 2, 1]
folded into the stage-1 DFT constants.  Full-plane sum of |Z| equals
the weighted sum over the half-plane.

Sharding: data-parallel over batch B=32 -> 4 items (36 diff images)
per NeuronCore.  Each core outputs per-image partial sums [128, 72]
(128 partitions x 2 v-chunks per image); the host finishes the tiny
reduction and the contrastive combine.
"""

import numpy as np

import concourse.bass as bass
import concourse.tile as tile
from concourse import bacc, mybir
from concourse.bass_utils import run_bass_kernel_spmd

B, C, H, W = 32, 3, 256, 256
NCORES = 8
BL = B // NCORES          # batch items per core
NIMG = BL * C * 3         # 36 difference images per core
NU = 130                  # u columns: u=0..128 real + 1 zero pad col (even width)
NF = 2 * NU               # [real | imag] packed free dim
EPS = 1e-7
F16 = mybir.dt.float16
F32 = mybir.dt.float32


def _build_consts():
    """DFT constant matrices (fp16) with symmetry weights folded in."""
    r = np.arange(H, dtype=np.float64)
    u = np.arange(NU, dtype=np.float64)
    w = np.ones(NU)
    w[1:128] = 2.0
    w[129] = 0.0                      # dummy pad column
    ang1 = 2.0 * np.pi * np.outer(r, u) / H       # [row, u]
    frw = np.cos(ang1) * w
    fiw = -np.sin(ang1) * w
    frw[:, 129] = 0.0
    fiw[:, 129] = 0.0
    fab = np.concatenate([frw, fiw], axis=1)      # [256, 260]
    fab = fab.reshape(2, 128, NF).astype(np.float16)

    ang2 = 2.0 * np.pi * np.outer(r, r) / H       # [col, v]
    fr2 = np.cos(ang2).reshape(2, 128, W).astype(np.float16)
    fi2 = (-np.sin(ang2)).reshape(2, 128, W).astype(np.float16)
    return fab, fr2, fi2


def _build_program(reps=1):
    """Bass/Tile program: 36 images x (2-stage DFT matmul + |Z| reduce).

    reps>1 repeats the whole image loop (identical output) — used only
    by the benchmark harness to measure steady-state per-rep HW time.
    """
    nc = bacc.Bacc("TRN2", debug=False, num_devices=NCORES)
    x_d = nc.dram_tensor("x", [NIMG, 2, 128, W], F16, kind="ExternalInput").ap()
    fab_d = nc.dram_tensor("fab", [2, 128, NF], F16, kind="ExternalInput").ap()
    fr2_d = nc.dram_tensor("fr2", [2, 128, W], F16, kind="ExternalInput").ap()
    fi2_d = nc.dram_tensor("fi2", [2, 128, W], F16, kind="ExternalInput").ap()
    out_d = nc.dram_tensor("out", [128, NIMG], F32, kind="ExternalOutput").ap()

    with tile.TileContext(nc) as tc:
        with (
            tc.tile_pool(name="consts", bufs=1) as cpool,
            tc.tile_pool(name="xin", bufs=3) as xpool,
            tc.tile_pool(name="ysb", bufs=3) as ypool,
            tc.tile_pool(name="tmp", bufs=3) as tpool,
            tc.tile_pool(name="accp", bufs=1) as apool,
            tc.tile_pool(name="py", bufs=2, space="PSUM") as pypool,
            tc.tile_pool(name="pz", bufs=2, space="PSUM") as pzpool,
        ):
            fab = [cpool.tile([128, NF], F16, tag=f"fab{k}", name=f"fab{k}") for k in range(2)]
            fr2 = [cpool.tile([128, W], F16, tag=f"fr2{k}", name=f"fr2{k}") for k in range(2)]
            fi2 = [cpool.tile([128, W], F16, tag=f"fi2{k}", name=f"fi2{k}") for k in range(2)]
            for k in range(2):
                nc.sync.dma_start(out=fab[k], in_=fab_d[k])
                nc.sync.dma_start(out=fr2[k], in_=fr2_d[k])
                nc.sync.dma_start(out=fi2[k], in_=fi2_d[k])
            acc = apool.tile([128, NIMG], F32)

            for j in [jj for _ in range(reps) for jj in range(NIMG)]:
                xs = [xpool.tile([128, W], F16, tag=f"x{k}", name=f"x{k}") for k in range(2)]
                for k in range(2):
                    nc.sync.dma_start(out=xs[k], in_=x_d[j, k])

                # Stage 1: yT[col, u] = sum_row x[row, col] * F[row, u]
                # y psum = [yr | yi], built per col-chunk m.
                ysb, ysb2 = [], []
                for m in range(2):
                    y = pypool.tile([128, NF], F32, tag=f"y{m}")
                    for k in range(2):
                        nc.tensor.matmul(
                            y,
                            lhsT=xs[k][:, m * 128 : (m + 1) * 128],
                            rhs=fab[k],
                            start=(k == 0),
                            stop=(k == 1),
                        )
                    ys = ypool.tile([128, NF], F16, tag=f"ysb{m}")
                    nc.vector.tensor_copy(ys, y)          # fp32 -> fp16 cast
                    ys2 = ypool.tile([128, NF], F16, tag=f"ysb2{m}")
                    nc.vector.tensor_scalar_mul(ys2[:, 0:NU], ys[:, NU:NF], -1.0)
                    nc.vector.tensor_copy(ys2[:, NU:NF], ys[:, 0:NU])
                    ysb.append(ys)
                    ysb2.append(ys2)

                # Stage 2: zT[v, u] = sum_col F[col, v] * y[col, u]
                # z psum = [zr | zi]:
                #   lhsT=Fr, rhs=[yr|yi]   -> [Fr.yr | Fr.yi]
                #   lhsT=Fi, rhs=[-yi|yr]  -> [-Fi.yi | Fi.yr]
                tbig = tpool.tile([128, 4 * NU], F32, tag="tbig")
                for v in range(2):
                    z = pzpool.tile([128, NF], F32, tag=f"z{v}")
                    vs = slice(v * 128, (v + 1) * 128)
                    nc.tensor.matmul(z, lhsT=fr2[0][:, vs], rhs=ysb[0],
                                     start=True, stop=False)
                    nc.tensor.matmul(z, lhsT=fr2[1][:, vs], rhs=ysb[1],
                                     start=False, stop=False)
                    nc.tensor.matmul(z, lhsT=fi2[0][:, vs], rhs=ysb2[0],
                                     start=False, stop=False)
                    nc.tensor.matmul(z, lhsT=fi2[1][:, vs], rhs=ysb2[1],
                                     start=False, stop=True)
                    nc.scalar.activation(
                        out=tbig[:, v * NF : (v + 1) * NF], in_=z,
                        func=mybir.ActivationFunctionType.Square,
                    )

                # |Z| = sqrt(zr^2 + zi^2) over both v chunks in one pass,
                # summed over the free dim into one acc column per image.
                t4 = tbig.rearrange("p (a b) -> p a b", b=NU)
                s = tpool.tile([128, 2 * NU], F32, tag="s")
                nc.gpsimd.tensor_add(
                    s.rearrange("p (a b) -> p a b", b=NU),
                    t4[:, 0:4:2, :],
                    t4[:, 1:4:2, :],
                )
                sq = tpool.tile([128, 2 * NU], F32, tag="sq")
                nc.scalar.activation(
                    out=sq,
                    in_=s,
                    func=mybir.ActivationFunctionType.Sqrt,
                    accum_out=acc[:, j : j + 1],
                )

            nc.sync.dma_start(out=out_d, in_=acc)

    nc.compile()
    return nc


_CACHE = {}


def _get_program():
    if "nc" not in _CACHE:
        _CACHE["nc"] = _build_program()
    return _CACHE["nc"]


def _prep_core_inputs(a, p, n, neg_idx, fab, fr2, fi2):
    """Shard over batch; build the 36 fp16 difference images per core."""
    neg = neg_idx.astype(np.int64)
    in_maps = []
    for c in range(NCORES):
        sl = slice(c * BL, (c + 1) * BL)
        ash = a[sl]                                   # [BL, C, H, W]
        d0 = ash - p[sl]
        d1 = ash - n[neg[sl, 0]]
        d2 = ash - n[neg[sl, 1]]
        diffs = np.stack([d0, d1, d2], axis=2)        # [BL, C, 3, H, W]
        x = diffs.reshape(NIMG, 2, 128, W).astype(np.float16)
        in_maps.append({"x": x, "fab": fab, "fr2": fr2, "fi2": fi2})
    return in_maps


def run(a, p, n, neg_idx, trace=False, trace_kwargs=None):
    """Run on 8 cores; returns (scalar_output, BassKernelResults)."""
    nc = _get_program()
    fab, fr2, fi2 = _build_consts()
    in_maps = _prep_core_inputs(
        np.asarray(a, dtype=np.float32),
        np.asarray(p, dtype=np.float32),
        np.asarray(n, dtype=np.float32),
        np.asarray(neg_idx),
        fab, fr2, fi2,
    )
    last_err = None
    for _attempt in range(3):
        try:
            res = run_bass_kernel_spmd(
                nc, in_maps, core_ids=list(range(NCORES)),
                trace=trace, **(trace_kwargs or {}),
            )
            break
        except Exception as e:  # rare transient device faults -> retry
            last_err = e
    else:
        raise last_err

    total = 0.0
    norm = float(C * H * W)
    for c in range(NCORES):
        acc = res.results[c]["out"]                   # [128, 36] fp32
        s_img = acc.sum(axis=0, dtype=np.float64)     # [36]
        d = s_img.reshape(BL, C, 3).sum(axis=1) / norm  # [BL, 3]
        d_ap, d_an0, d_an1 = d[:, 0], d[:, 1], d[:, 2]
        total += (d_ap / (d_an0 + EPS) + d_ap / (d_an1 + EPS)).sum()
    out = np.float32(total / (2 * B))
    return np.asarray(out, dtype=np.float32), res


def kernel(a, p, n, neg_idx):
    out, _ = run(a, p, n, neg_idx)
    return out
